# revision 10
# baseline (speedup 1.0000x reference)
"""DigitCaps dynamic-routing kernel for 8x TRN2 NeuronCores (Bass/Tile).

Reference math (per sample b, capsule j):
    u_hat[b,j,r,o] = sum_i W[j,r,o,i] * u[b,r,i]
    b_ij = 0
    3 iterations:
        c = softmax_r(b_ij); s = sum_r c*u_hat; v = squash(s)
        b_ij += sum_o u_hat*v  (first 2 iters)
    return v  [B, J, O]

Sharding: J (166 -> 168 padded) split across 8 cores, 21 capsules each.
Every core sees the full batch B=128 (partition dim for batch-parallel ops).

Per-core algorithm (u_hat never materialized; all heavy contractions on PE):
    s0    = sum_{r,i} (W/R)[j,r,o,i] u[b,r,i]            (PE, k=(r,i) chunks)
    v0    = squash(s0)                                    (tiny DVE/ACT)
    iter t=1,2:
      T1[b,(r,i)] = sum_o v[b,j,o] W[j,r,o,i]            (PE, k=o, per j)
      a[b,j,r]    = sum_i T1*u                            (DVE mul+reduce)
      b_ij        = a1 (+ a2 on t=2, via DRAM scratch)
      e = exp(b_ij) in r-on-partition layout              (DMA-transpose + ACT)
      Z = sum_r e                                         (PE ones-matmul)
      cur[(r),(i,b)] = e * u                              (DVE)
      sT[o,(j,b)] = sum_{r,i} W*cur                       (PE, k=r chunks)
      v = squash(sT / Z)                                  (o-partition layout)
"""

import sys

sys.path.insert(0, "/opt/trn_rl_repo")

import numpy as np
import ml_dtypes
from contextlib import ExitStack

import concourse.bacc as bacc
import concourse.bass as bass
import concourse.tile as tile
from concourse import mybir
from concourse.masks import make_identity
from concourse.bass_utils import run_bass_kernel_spmd

F32 = mybir.dt.float32
BF16 = mybir.dt.bfloat16
AX = mybir.AxisListType
ACT_F = mybir.ActivationFunctionType

B, J, R, O, I = 128, 166, 864, 8, 4
NCORES = 8
JL = 21          # capsules per core (166 padded to 168)
RP = 896         # R padded to 7*128
C32 = 27         # (r32, i4)=128 contraction chunks over real R=864
C128 = 7         # r chunks of 128 over RP
JO = JL * O      # 168
JB = JL * B      # 2688
NZB = 6          # Z col-blocks of 448 (6*448 = 2688)
ZW = 448


def build_nc():
    nc = bacc.Bacc(
        "TRN2", target_bir_lowering=False, debug=False, enable_asserts=False
    )

    d_uT_ri = nc.dram_tensor("uT_ri", [C32, 128, 128], F32, kind="ExternalInput")
    d_Wm_ri = nc.dram_tensor("Wm_ri", [C32, 128, JO], F32, kind="ExternalInput")
    d_Wr = nc.dram_tensor("Wr", [C128, 128, I * JL * O], BF16, kind="ExternalInput")
    d_W2 = nc.dram_tensor("W2", [JL, 8, I * RP], BF16, kind="ExternalInput")
    d_u_b = nc.dram_tensor("u_b", [128, RP * I], F32, kind="ExternalInput")
    d_uTr = nc.dram_tensor("uTr", [C128, 128, I * B], BF16, kind="ExternalInput")
    d_out = nc.dram_tensor("v_out", [8, JL, B], F32, kind="ExternalOutput")

    with tile.TileContext(nc) as tc:
        _body(tc, d_uT_ri, d_Wm_ri, d_Wr, d_W2, d_u_b, d_uTr, d_out)
    nc.compile()
    return nc


def _body(tc, d_uT_ri, d_Wm_ri, d_Wr, d_W2, d_u_b, d_uTr, d_out):
    nc = tc.nc
    es = ExitStack()
    const = es.enter_context(tc.tile_pool(name="const", bufs=1))
    misc = es.enter_context(tc.tile_pool(name="misc", bufs=1))
    vpool = es.enter_context(tc.tile_pool(name="vpool", bufs=2))
    stream = es.enter_context(tc.tile_pool(name="stream", bufs=2))
    prodp = es.enter_context(tc.tile_pool(name="prodp", bufs=3))
    curp = es.enter_context(tc.tile_pool(name="curp", bufs=8))
    ep = es.enter_context(tc.tile_pool(name="ep", bufs=1))
    dramp = es.enter_context(tc.tile_pool(name="dramp", bufs=1, space="DRAM"))
    psum_big = es.enter_context(tc.tile_pool(name="psum_big", bufs=1, space="PSUM"))
    psum_sm = es.enter_context(tc.tile_pool(name="psum_sm", bufs=1, space="PSUM"))

    with es:
        # ---------------- constants / persistent loads ----------------
        identity = const.tile([128, 128], F32)
        make_identity(nc, identity[:])
        ones_r = const.tile([128, 1], BF16)
        nc.vector.memset(ones_r[:], 1.0)
        ones_r96 = const.tile([128, 1], BF16)
        nc.vector.memset(ones_r96[:], 0.0)
        nc.vector.memset(ones_r96[:96, :], 1.0)
        ones_o = const.tile([8, 1], BF16)
        nc.vector.memset(ones_o[:], 1.0)

        u_b = const.tile([128, RP * I], F32)
        nc.sync.dma_start(u_b[:], d_u_b[:])
        uTr = []
        for c in range(C128):
            t = const.tile([128, I * B], BF16, tag=f"uTr{c}", name=f"uTr{c}")
            nc.sync.dma_start(t[:], d_uTr[c])
            uTr.append(t)
        wr = []
        for c in range(C128):
            t = const.tile([128, I * JL * O], BF16, tag=f"wr{c}", name=f"wr{c}")
            nc.sync.dma_start(t[:], d_Wr[c])
            wr.append(t)

        a1_dram = dramp.tile([JL, 128, RP], F32)
        g_dram = dramp.tile([1, JB], F32)

        # ---------------- s0: full (r,i) contraction, b-partition out -------
        with tc.tile_pool(name="s0p", bufs=4) as s0p:
            s0ps = psum_sm.tile([128, 512], F32, tag="small", name="s0ps")
            for c in range(C32):
                a = s0p.tile([128, 128], F32, tag="ut", name=f"ut{c}")
                nc.sync.dma_start(a[:], d_uT_ri[c])
                b = s0p.tile([128, JO], F32, tag="wm", name=f"wm{c}")
                nc.sync.dma_start(b[:], d_Wm_ri[c])
                nc.tensor.matmul(
                    s0ps[:, :JO],
                    lhsT=a[:],
                    rhs=b[:],
                    start=(c == 0),
                    stop=(c == C32 - 1),
                )
            s0_sb = s0p.tile([128, JO], F32)
            nc.scalar.copy(s0_sb[:], s0ps[:, :JO])

            # squash in b-layout (tiny)
            sqb = s0p.tile([128, JO], F32)
            nc.vector.tensor_mul(sqb[:], s0_sb[:], s0_sb[:])
            n2b = s0p.tile([128, JL], F32)
            nc.vector.reduce_sum(
                n2b[:], sqb[:].rearrange("p (j o) -> p j o", j=JL), axis=AX.X
            )
            rtb = s0p.tile([128, JL], F32)
            nc.scalar.sqrt(rtb[:], n2b[:])
            nc.vector.tensor_scalar_add(n2b[:], n2b[:], 1.0)
            nc.vector.reciprocal(n2b[:], n2b[:])
            nc.vector.tensor_mul(rtb[:], rtb[:], n2b[:])  # f = sqrt(n2)/(1+n2)
            v0b = s0p.tile([128, JO], F32)
            nc.vector.tensor_mul(
                v0b[:].rearrange("p (j o) -> p j o", j=JL),
                s0_sb[:].rearrange("p (j o) -> p j o", j=JL),
                rtb[:].unsqueeze(2).broadcast_to([128, JL, O]),
            )
            # transpose v0 to o-partition layout -> v_prev (bf16)
            v_prev = vpool.tile([8, JB], BF16, tag="vbf")
            jj = 0
            while jj < JL:
                take = min(4, JL - jj)
                tps = psum_sm.tile([128, 512], F32, tag="small", name="tps")
                for q in range(take):
                    nc.tensor.transpose(
                        tps[0:8, q * 128 : (q + 1) * 128],
                        v0b[:].rearrange("p (j o) -> p j o", j=JL)[:, jj + q, :],
                        identity[:],
                    )
                nc.scalar.copy(
                    v_prev[:, jj * 128 : (jj + take) * 128], tps[0:8, : take * 128]
                )
                jj += take

        # ---------------- routing iterations ----------------
        for t in (1, 2):
            e_t = [ep.tile([128, JB], BF16, tag=f"e{c}", name=f"e_{t}_{c}") for c in range(C128)]

            # ---- a-phase: per-j T1 matmul + mul/reduce + transpose out ----
            for j in range(JL):
                w2 = stream.tile([8, I * RP], BF16, tag="w2", name="w2")
                nc.sync.dma_start(w2[:], d_W2[j])
                w2v = w2[:].rearrange("p (i r) -> p i r", i=I)
                t1 = psum_big.tile([128, 512 * C128], F32, tag="T1", name="t1")
                lhs = v_prev[:, j * 128 : (j + 1) * 128]
                for c in range(C128):
                    mv = w2v[:, :, c * 128 : (c + 1) * 128].transpose([0, 2, 1])
                    nc.tensor.matmul(
                        t1[:, c * 512 : (c + 1) * 512],
                        lhsT=lhs,
                        rhs=mv,
                        start=True,
                        stop=True,
                    )
                a_j = stream.tile([128, RP], F32, tag="aj", name="a_j")
                for c in range(C128):
                    prod = prodp.tile([128, 512], BF16, tag="prod", name="prod")
                    nc.vector.tensor_mul(
                        prod[:],
                        t1[:, c * 512 : (c + 1) * 512],
                        u_b[:, c * 512 : (c + 1) * 512],
                    )
                    nc.vector.reduce_sum(
                        a_j[:, c * 128 : (c + 1) * 128],
                        prod[:].rearrange("p (r i) -> p r i", i=I),
                        axis=AX.X,
                    )
                a_bf = stream.tile([128, RP], BF16, tag="abf", name="a_bf")
                if t == 1:
                    nc.scalar.copy(a_bf[:], a_j[:])
                    nc.sync.dma_start(a1_dram[j], a_j[:])
                else:
                    a1_f = stream.tile([128, RP], F32, tag="a1f", name="a1_f")
                    nc.sync.dma_start(a1_f[:], a1_dram[j])
                    nc.vector.tensor_add(a_bf[:], a_j[:], a1_f[:])
                for c in range(C128):
                    nc.scalar.dma_start_transpose(
                        e_t[c][:, j * 128 : (j + 1) * 128],
                        a_bf[:, c * 128 : (c + 1) * 128],
                    )

            # ---- e-phase: exp in place; Z = sum_r e; rZ = 1/Z ----
            for c in range(C128):
                nc.scalar.activation(e_t[c][:], e_t[c][:], ACT_F.Exp)
            rZ = misc.tile([1, JB], F32, tag="rZ", name=f"rZ_{t}")
            for n in range(NZB):
                zs = psum_sm.tile([128, 512], F32, tag="small", name="zs")
                for c in range(C128):
                    ones = ones_r96 if c == C128 - 1 else ones_r
                    nc.tensor.matmul(
                        zs[0:1, :ZW],
                        lhsT=ones[:],
                        rhs=e_t[c][:, n * ZW : (n + 1) * ZW],
                        start=(c == 0),
                        stop=(c == C128 - 1),
                    )
                nc.vector.reciprocal(zs[0:1, :ZW], zs[0:1, :ZW])
                nc.scalar.copy(rZ[:, n * ZW : (n + 1) * ZW], zs[0:1, :ZW])

            # ---- s-phase: cur = e*u per (c,j); sT = sum_{r,i} W*cur ----
            sT = misc.tile([8, JB], F32, tag="sT", name=f"sT_{t}")
            jj = 0
            while jj < JL:
                take = min(4, JL - jj)
                sps = psum_sm.tile([128, 512], F32, tag="small", name="sps")
                for q in range(take):
                    jx = jj + q
                    curs = []
                    for c in range(C128):
                        cur = curp.tile([128, I * B], BF16, tag="cur", name="cur")
                        nc.vector.tensor_mul(
                            cur[:].rearrange("p (i b) -> p i b", i=I),
                            e_t[c][:, jx * 128 : (jx + 1) * 128]
                            .unsqueeze(1)
                            .broadcast_to([128, I, B]),
                            uTr[c][:].rearrange("p (i b) -> p i b", i=I),
                        )
                        curs.append(cur)
                    for c in range(C128):
                        for i in range(I):
                            nc.tensor.matmul(
                                sps[0:8, q * 128 : (q + 1) * 128],
                                lhsT=wr[c][:, (i * JL + jx) * 8 : (i * JL + jx) * 8 + 8],
                                rhs=curs[c][:].rearrange("p (i b) -> p i b", i=I)[
                                    :, i, :
                                ],
                                start=(c == 0 and i == 0),
                                stop=(c == C128 - 1 and i == I - 1),
                            )
                nc.scalar.copy(sT[:, jj * 128 : (jj + take) * 128], sps[0:8, : take * 128])
                jj += take

            # ---- squash phase in o-partition layout ----
            # v = sT * g,  g = rZ*sqrt(q)/(1+q),  q = rZ^2 * m2,  m2 = sum_o sT^2
            sq = misc.tile([8, JB], BF16, tag="sq", name=f"sq_{t}")
            nc.scalar.square(sq[:], sT[:])
            m2 = misc.tile([1, JB], F32, tag="m2", name=f"m2_{t}")
            for n in range(NZB):
                ns = psum_sm.tile([128, 512], F32, tag="small", name="ns")
                nc.tensor.matmul(
                    ns[0:1, :ZW],
                    lhsT=ones_o[:],
                    rhs=sq[:, n * ZW : (n + 1) * ZW],
                    start=True,
                    stop=True,
                )
                nc.scalar.copy(m2[:, n * ZW : (n + 1) * ZW], ns[0:1, :ZW])
            tmp = misc.tile([1, JB], F32, tag="tmp", name=f"tmp_{t}")
            nc.vector.tensor_mul(tmp[:], rZ[:], rZ[:])
            nc.vector.tensor_mul(m2[:], m2[:], tmp[:])  # q
            nc.scalar.sqrt(tmp[:], m2[:])               # sqrt(q)
            nc.vector.tensor_scalar_add(m2[:], m2[:], 1.0)
            nc.vector.reciprocal(m2[:], m2[:])          # 1/(1+q)
            nc.vector.tensor_mul(tmp[:], tmp[:], m2[:])
            nc.vector.tensor_mul(tmp[:], tmp[:], rZ[:])  # g
            g8 = misc.tile([8, JB], F32, tag="g8", name=f"g8_{t}")
            nc.gpsimd.dma_start(g_dram[:], tmp[:])
            nc.gpsimd.dma_start(
                g8[:], g_dram[:].squeeze(0).unsqueeze(0).broadcast_to([8, JB])
            )

            if t == 1:
                v_prev = vpool.tile([8, JB], BF16, tag="vbf", name="v1")
                nc.vector.tensor_mul(v_prev[:], sT[:], g8[:])
            else:
                v_fin = misc.tile([8, JB], F32, tag="vfin", name="v_fin")
                nc.vector.tensor_mul(v_fin[:], sT[:], g8[:])
                nc.sync.dma_start(d_out[:].rearrange("o j b -> o (j b)"), v_fin[:])


# ---------------------------------------------------------------------------
# Host side
# ---------------------------------------------------------------------------

_NC_CACHE = None


def _get_nc():
    global _NC_CACHE
    if _NC_CACHE is None:
        _NC_CACHE = build_nc()
    return _NC_CACHE


def _host_prep(u, W):
    """Build per-core input maps. u: [B,R,I] f32; W: [1,J,R,O,I] f32."""
    bf = ml_dtypes.bfloat16
    u = np.ascontiguousarray(u, dtype=np.float32)
    Wq = np.ascontiguousarray(W.reshape(J, R, O, I), dtype=np.float32)

    up = np.zeros((B, RP, I), np.float32)
    up[:, :R] = u
    u_b = np.ascontiguousarray(up.reshape(128, RP * I))
    uT = np.ascontiguousarray(u.transpose(1, 2, 0))        # [864, 4, 128]
    uT_ri = np.ascontiguousarray(uT.reshape(C32, 128, 128))
    uTp = np.ascontiguousarray(up.transpose(1, 2, 0))      # [896, 4, 128]
    uTr = np.ascontiguousarray(uTp.reshape(C128, 128, I * B).astype(bf))

    in_maps = []
    for k in range(NCORES):
        j0 = k * JL
        Wk = np.zeros((JL, R, O, I), np.float32)
        real = min(JL, max(0, J - j0))
        if real > 0:
            Wk[:real] = Wq[j0 : j0 + real]
        Wkp = np.zeros((JL, RP, O, I), np.float32)
        Wkp[:, :R] = Wk

        wm_ri = np.ascontiguousarray(
            (Wk / float(R)).transpose(1, 3, 0, 2).reshape(C32, 128, JO)
        )
        wr = np.ascontiguousarray(
            Wkp.transpose(1, 3, 0, 2).reshape(C128, 128, I * JL * O).astype(bf)
        )
        w2 = np.ascontiguousarray(
            Wkp.transpose(0, 2, 3, 1).reshape(JL, 8, I * RP).astype(bf)
        )
        in_maps.append(
            {
                "uT_ri": uT_ri,
                "Wm_ri": wm_ri,
                "Wr": wr,
                "W2": w2,
                "u_b": u_b,
                "uTr": uTr,
            }
        )
    return in_maps


def run_cores(u, W, trace=False):
    nc = _get_nc()
    in_maps = _host_prep(u, W)
    res = run_bass_kernel_spmd(
        nc, in_maps, core_ids=list(range(NCORES)), trace=trace
    )
    return res


def kernel(u, W):
    res = run_cores(u, W, trace=False)
    parts = []
    for k in range(NCORES):
        vk = res.results[k]["v_out"]          # [8, JL, 128] (o, j, b)
        parts.append(np.ascontiguousarray(vk.transpose(2, 1, 0)))  # [B, JL, O]
    full = np.concatenate(parts, axis=1)[:, :J, :]
    return np.ascontiguousarray(full.astype(np.float32))
